# revision 30
# baseline (speedup 1.0000x reference)
"""BP LDPC decoder as an on-chip PE-routed Trainium2 kernel.

The reference multiplies dense [E,E] (E=3456) "exclusive sum" operators every
iteration.  Those operators are the check-node / variable-node exclusive sums
of a (DV=3)-regular LDPC graph.  The variable-node side is local to a
partition-major edge layout; the check-node side (a fixed graph permutation +
segmented reduce) is done entirely on the tensor engine with 0/1 routing
matrices held in SBUF:

  scatter: tot[pc, g, :] = sum_s A_s^T @ (pair[:, s] masked into its g slot)
  gather:  U[pe, s, g, :] = B_s^T @ tot ; G = sum_g U * mask

so nothing round-trips through DRAM inside the iteration loop (the baseline
did 54 indirect DMAs per iteration through HBM; that was ~90% of its time).

The routed channels are fp16 (fp32 matmuls run 2 PE passes; fp16 runs one and
gets fast weight load).  The log-magnitude channel is carried as an fp16
hi/lo pair (lo = lt - fp16(lt)), so the reconstructed check sums match fp32
to ~2^-22 relative -- the decoded bits stay bit-exact vs the fp32 reference.
The sign-count channel is exact (small integers).

Sharding: pure data parallel over the batch (16 -> 2 samples per core); the
graph structure (routing matrices) is replicated.
"""

import sys
import numpy as np

for _p in ("/opt/trn_rl_repo", "/root/.axon_site/_ro/trn_rl_repo"):
    if _p not in sys.path:
        sys.path.insert(0, _p)

N = 1152          # variables
E = 3456          # edges (DV=3 per variable)
B = 16            # batch
NCORES = 8
BP = B // NCORES  # batch per core
NQ = N // 128     # variables per partition (9)
NS = E // 128     # edge slots per partition (27)
NG = 5            # check groups of 128 (576 checks -> 5*128 slots)
RC = 3            # routed channels: (lt_hi, lt_lo, b)
FW = RC * BP      # matmul free width per (s, g)

_A32 = float(np.float32(1.0 + 1e-8))
_B32 = float(np.float32(1.0 - 1e-8))


def _derive_structure(H_sumC_to_V, H_xe_v_sumc_to_y):
    """Recover the LDPC graph and build the PE routing operators.

    Device edge order is col-major (variable-major): e = 3*v + j, variable v
    on partition v // NQ, slot s = e % NS.  Check c sits at PSUM row c % 128
    in group c // 128.
    Returns:
      wA [128, NS, 128] f16: wA[pe, s, pc] = 1 iff edge (pe,s) -> check row pc
      wB [128, NS, 128] f16: wB[pc, s, pe] = wA[pe, s, pc]   (gather routing)
      mk [128, NS, NG] f32:  mk[pe, s, g]  = 1 iff edge (pe,s) -> group g
    """
    H_sumC_to_V = np.asarray(H_sumC_to_V)
    H_xe_v_sumc_to_y = np.asarray(H_xe_v_sumc_to_y)
    cols_rm = np.argmax(H_xe_v_sumc_to_y, axis=0)        # variable of row-major edge
    p_r2l = np.argsort(cols_rm, kind="stable")           # col-major pos -> row-major idx
    p_l2r = np.argsort(p_r2l)
    Br = H_sumC_to_V[:, p_l2r]                           # same-check adjacency (row-major)
    same = Br[np.arange(E - 1), np.arange(1, E)] > 0
    check_id = np.concatenate([[0], np.cumsum(~same)]).astype(np.int64)
    ck = check_id[p_r2l]                                 # check of device edge e
    assert ck.max() < NG * 128
    pc = (ck % 128).astype(np.int64).reshape(128, NS)    # [pe, s]
    g = (ck // 128).astype(np.int64).reshape(128, NS)
    pe_idx = np.arange(128)[:, None].repeat(NS, 1)
    s_idx = np.arange(NS)[None, :].repeat(128, 0)
    wA = np.zeros((128, NS, 128), np.float16)
    wA[pe_idx, s_idx, pc] = 1.0
    wB = np.zeros((128, NS, 128), np.float16)
    wB[pc, s_idx, pe_idx] = 1.0
    mk = np.zeros((128, NS, 1, NG), np.float32)
    mk[pe_idx, s_idx, 0, g] = 1.0
    mkb = np.zeros((128, NS, NG, RC, BP), np.float16)
    mkb[pe_idx, s_idx, g] = 1.0
    return wA, wB, mk, mkb


def _build_program(n_iters: int):
    import concourse.bacc as bacc
    import concourse.hw_specs as hw_specs
    # Force every activation onto the one table set containing all our
    # functions (ln/exp/sign/abs); otherwise the chooser ping-pongs between
    # sets and reloads tables (~2.7us per reload).
    if not getattr(bacc, "_act_tables_pinned", False):
        _orig_get = hw_specs.get_activation_tables

        def _pinned(arch):
            tabs = _orig_get(arch)
            keep = "natural_log_exp_and_others"
            if keep in tabs:
                tabs = {k: (v if k == keep else set()) for k, v in tabs.items()}
            return tabs

        bacc.get_activation_tables = _pinned
        bacc._act_tables_pinned = True
    import concourse.mybir as mybir
    import concourse.tile as tile

    f32 = mybir.dt.float32
    f16 = mybir.dt.float16
    i32 = mybir.dt.int32
    AF = mybir.ActivationFunctionType
    ALU = mybir.AluOpType

    nc = bacc.Bacc("TRN2", target_bir_lowering=False, debug=False)

    llr_d = nc.declare_dram_parameter("llr", [BP, N], f32, isOutput=False)
    wA_d = nc.declare_dram_parameter("wA", [128, NS * 128], f16, isOutput=False)
    wB_d = nc.declare_dram_parameter("wB", [128, NS * 128], f16, isOutput=False)
    mkb_d = nc.declare_dram_parameter("mkb", [128, NS * NG * RC * BP], f16,
                                      isOutput=False)
    dec_d = nc.declare_dram_parameter("dec", [BP, N], i32, isOutput=True)

    with tile.TileContext(nc) as tc:
        with tc.tile_pool(name="st", bufs=1) as st, \
             tc.tile_pool(name="ps", bufs=1, space="PSUM") as ps:
            wA = st.tile([128, NS, 128], f16)
            wB = st.tile([128, NS, 128], f16)
            mk = st.tile([128, NS, 1, NG], f32)
            mkb = st.tile([128, NS, NG, RC, BP], f16)  # pre-broadcast fp16 mask
            llr_sb = st.tile([128, NQ, BP], f32)
            x = st.tile([128, NQ, 3, BP], f32)         # per-edge v->c messages
            lt_f = st.tile([128, NS, BP], f32)         # fp32 log-magnitude
            pair = st.tile([128, NS, RC, BP], f16)     # (lt_hi, lt_lo, b) fp16
            R = st.tile([128, NS, NG, RC, BP], f16)    # mask-expanded rhs
            tbf = st.tile([128, RC, BP, NG], f16)      # gather rhs (hi, lo, K)
            Lt_f = st.tile([128, NG, BP], f32)
            Um = st.tile([128, NS, RC, BP, NG], f32)
            G3 = st.tile([128, NS, RC, BP], f32)       # gathered (hi, lo, K) sums
            S = st.tile([128, NQ, 3, BP], f32)
            dlt = st.tile([128, NQ, 3, BP], f32)
            db = st.tile([128, NQ, 3, BP], f32)
            lden = st.tile([128, NQ, 3, BP], f32)
            xp = st.tile([128, NQ, 3, BP], f32)
            ki = st.tile([128, NQ, 3, BP], i32)
            kb = st.tile([128, NQ, 3, BP], i32)
            sgnx = st.tile([128, NQ, 3, BP], f32)
            w = st.tile([128, NQ, 3, BP], f32)
            pd = st.tile([128, NQ, 3, BP], f32)
            L1 = st.tile([128, NQ, 3, BP], f32)
            L2 = st.tile([128, NQ, 3, BP], f32)
            yv = st.tile([128, NQ, 4, BP], f32)        # slots 0..2 = y_j, 3 = llr
            Vf = st.tile([128, NQ, BP], f32)           # llr + sum_j y_j
            sg = st.tile([128, NQ, BP], f32)
            dec_f = st.tile([128, NQ, BP], f32)
            dec_i = st.tile([128, NQ, BP], i32)

            tot_ps = ps.tile([128, NG, RC, BP], f32)   # 120B -> 1 bank
            # gather output in THREE tiles so the mask-select starts while the
            # PE is still filling later tiles (deps are per-tile)
            U_a = ps.tile([128, 10, 32], f32)          # s 0..9
            U_b = ps.tile([128, 9, 32], f32)           # s 10..18
            U_c = ps.tile([128, NS - 19, 32], f32)     # s 19..26
            ax_ps = ps.tile([128, NQ, 3, BP], f32)
            u_ps = ps.tile([128, NQ, 3, BP], f32)
            ln_ps = ps.tile([128, NQ, 3, BP], f32)

            # ---- init (ordered by when iteration 0 needs each tensor) ----
            nc.sync.dma_start(
                out=llr_sb[:], in_=llr_d.ap().rearrange("b (p q) -> p q b", p=128)
            )
            wA_ap = wA_d.ap().rearrange("p (s c) -> p s c", s=NS)
            wB_ap = wB_d.ap().rearrange("p (s c) -> p s c", s=NS)
            nc.sync.dma_start(out=wA[:, 0:14], in_=wA_ap[:, 0:14])
            nc.sync.dma_start(out=mkb[:], in_=mkb_d.ap())
            nc.sync.dma_start(out=wA[:, 14:NS], in_=wA_ap[:, 14:NS])
            nc.sync.dma_start(out=wB[:, 0:14], in_=wB_ap[:, 0:14])
            nc.sync.dma_start(out=wB[:, 14:NS], in_=wB_ap[:, 14:NS])
            # f32 mask for the PSUM-side select, derived from the fp16 one
            nc.vector.tensor_copy(mk[:, :, 0, :], mkb[:, :, :, 0, 0])
            nc.vector.tensor_copy(
                x[:], llr_sb[:].unsqueeze(2).to_broadcast([128, NQ, 3, BP])
            )
            nc.vector.tensor_copy(yv[:, :, 3, :], llr_sb[:])

            pair_hi = pair[:, :, 0, :]
            pair_lo = pair[:, :, 1, :]
            pair_b = pair[:, :, 2, :]
            ltq = lt_f[:].rearrange("p (q j) b -> p q j b", q=NQ)

            for t in range(n_iters):
                # lt = ln(1e-8 + tanh(|x|/2)) computed exp/ln-only:
                #   u = exp(-|x|); lt = ln(A - B*u) - ln(1 + u)
                # (intermediates go through PSUM: ScalarE's faster port)
                nc.scalar.activation(ax_ps[:], x[:], AF.Abs)
                nc.scalar.activation(u_ps[:], ax_ps[:], AF.Exp, scale=-1.0)
                nc.scalar.activation(ln_ps[:], u_ps[:], AF.Ln, bias=_A32,
                                     scale=-_B32)
                nc.scalar.activation(lden[:], u_ps[:], AF.Ln, bias=1.0)
                nc.vector.tensor_tensor(ltq, ln_ps[:], lden[:], ALU.subtract)
                # fp16 hi/lo split of lt; b = 1 if x < 0 else 0
                nc.vector.tensor_copy(pair_hi, lt_f[:])
                nc.vector.tensor_tensor(pair_lo, lt_f[:], pair_hi, ALU.subtract)
                nc.vector.tensor_scalar(
                    pair_b.rearrange("p (q j) b -> p q j b", q=NQ),
                    x[:], 0.0, None, ALU.is_lt)

                # mask-expand the per-edge rows into their check-group slot
                # (two halves so the scatter matmuls start on half A early)
                for sl, sh in ((0, 14), (14, NS)):
                    nc.vector.tensor_tensor(
                        R[:, sl:sh],
                        pair[:, sl:sh].unsqueeze(2).to_broadcast(
                            [128, sh - sl, NG, RC, BP]),
                        mkb[:, sl:sh],
                        ALU.mult)
                # scatter: tot[pc, g] = sum over edges of check (pc, g)
                for s in range(NS):
                    nc.tensor.matmul(
                        tot_ps[:],
                        wA[:, s, :],
                        R[:, s].rearrange("p g r b -> p (g r b)"),
                        start=(s == 0), stop=(s == NS - 1),
                    )
                # rebuild fp16 hi/lo of the per-check sums for the gather pass
                nc.vector.tensor_reduce(
                    Lt_f[:], tot_ps[:, :, 0:2, :].transpose([0, 1, 3, 2]),
                    axis=mybir.AxisListType.X, op=ALU.add)
                nc.vector.tensor_copy(
                    tbf[:, 0, :, :], Lt_f[:].transpose([0, 2, 1]))
                nc.vector.tensor_tensor(
                    tbf[:, 1, :, :], Lt_f[:].transpose([0, 2, 1]),
                    tbf[:, 0, :, :], ALU.subtract)
                nc.vector.tensor_copy(
                    tbf[:, 2, :, :], tot_ps[:, :, 2, :].transpose([0, 2, 1]))
                # gather: U[pe, s, g] = tot[pc(pe,s), g]  (free order r, b, g)
                tot_flat = tbf[:].rearrange("p r b g -> p (r b g)")
                uparts = ((U_a, 0, 10), (U_b, 10, 19), (U_c, 19, NS))
                for Ut, sl, sh in uparts:
                    for s in range(sl, sh):
                        nc.tensor.matmul(
                            Ut[:, s - sl, 0:NG * FW], wB[:, s, :], tot_flat,
                            start=True, stop=True,
                        )
                Um4 = Um[:].rearrange("p s r b g -> p s (r b) g")
                for Ut, sl, sh in uparts:
                    nc.vector.tensor_tensor(
                        Um4[:, sl:sh],
                        Ut[:, :, 0:NG * FW].rearrange(
                            "p s (rb g) -> p s rb g", g=NG),
                        mk[:, sl:sh].to_broadcast([128, sh - sl, RC * BP, NG]),
                        ALU.mult)
                nc.vector.tensor_reduce(
                    G3[:], Um4, axis=mybir.AxisListType.X, op=ALU.add)

                # exclusive check sums: sr = Lt - lt, kx = K - b
                G3q = G3[:].rearrange("p (q j) r b -> p q j r b", q=NQ)
                nc.vector.tensor_tensor(S[:], G3q[:, :, :, 0, :], G3q[:, :, :, 1, :],
                                        ALU.add)
                nc.vector.tensor_tensor(dlt[:], S[:], ltq, ALU.subtract)
                nc.vector.tensor_tensor(
                    db[:], G3q[:, :, :, 2, :],
                    pair_b.rearrange("p (q j) b -> p q j b", q=NQ), ALU.subtract)
                nc.scalar.activation(xp[:], dlt[:], AF.Exp)
                # sign of exclusive product: (-1)^kx
                nc.vector.tensor_copy(ki[:], db[:])
                nc.vector.tensor_scalar(kb[:], ki[:], 1, None, ALU.bitwise_and)
                nc.vector.tensor_scalar(sgnx[:], kb[:], -2.0, 1.0, ALU.mult, ALU.add)
                # y = ln(1 + pd) - ln(1 - pd), pd = sgn * (min(xp, 1) - 2e-7)
                # (clamp: fp16 hi/lo transport noise can push xp past 1 +- the
                #  2e-7 guard; the exact product never exceeds 1)
                nc.vector.tensor_scalar(w[:], xp[:], 1.0, -2e-7, ALU.min, ALU.add)
                nc.vector.tensor_tensor(pd[:], w[:], sgnx[:], ALU.mult)
                nc.scalar.activation(L1[:], pd[:], AF.Ln, bias=1.0)
                nc.scalar.activation(L2[:], pd[:], AF.Ln, bias=1.0, scale=-1.0)
                nc.vector.tensor_tensor(yv[:, :, 0:3, :], L1[:], L2[:], ALU.subtract)

                # variable side is local: Vf = llr + sum_j y_j
                nc.vector.tensor_reduce(
                    Vf[:], yv[:].transpose([0, 1, 3, 2]),
                    axis=mybir.AxisListType.X, op=ALU.add)
                if t < n_iters - 1:
                    nc.vector.tensor_tensor(
                        x[:],
                        Vf[:].unsqueeze(2).to_broadcast([128, NQ, 3, BP]),
                        yv[:, :, 0:3, :], ALU.subtract)
                else:
                    nc.scalar.activation(sg[:], Vf[:], AF.Sign)
                    nc.vector.tensor_scalar(dec_f[:], sg[:], -0.5, 0.5,
                                            ALU.mult, ALU.add)
                    nc.vector.tensor_copy(dec_i[:], dec_f[:])
                    nc.sync.dma_start(
                        out=dec_d.ap().rearrange("b (p q) -> p q b", p=128),
                        in_=dec_i[:],
                    )
    nc.compile()
    return nc


_PROGRAM_CACHE = {}


def _get_program(n_iters: int):
    if n_iters not in _PROGRAM_CACHE:
        _PROGRAM_CACHE[n_iters] = _build_program(n_iters)
    return _PROGRAM_CACHE[n_iters]


def _make_in_maps(llr_in, H_sumC_to_V, H_xe_v_sumc_to_y):
    llr = np.ascontiguousarray(np.asarray(llr_in, dtype=np.float32))
    assert llr.shape == (B, N)
    wA, wB, mk, mkb = _derive_structure(H_sumC_to_V, H_xe_v_sumc_to_y)
    wA = np.ascontiguousarray(wA.reshape(128, NS * 128))
    wB = np.ascontiguousarray(wB.reshape(128, NS * 128))
    mkb = np.ascontiguousarray(mkb.reshape(128, NS * NG * RC * BP))
    return [
        {
            "llr": np.ascontiguousarray(llr[c * BP:(c + 1) * BP]),
            "wA": wA,
            "wB": wB,
            "mkb": mkb,
        }
        for c in range(NCORES)
    ]


def kernel(llr_in, H_x_to_xe0, H_sumC_to_V, H_sumV_to_C, H_xe_v_sumc_to_y,
           bp_iter_num, **_unused):
    from concourse.bass_utils import run_bass_kernel_spmd

    n_iters = int(np.asarray(bp_iter_num))
    nc = _get_program(n_iters)
    in_maps = _make_in_maps(llr_in, H_sumC_to_V, H_xe_v_sumc_to_y)
    res = run_bass_kernel_spmd(nc, in_maps, list(range(NCORES)))
    out = np.concatenate([res.results[c]["dec"] for c in range(NCORES)], axis=0)
    return out.astype(np.int32)


# revision 33
# speedup vs baseline: 1.0191x; 1.0191x over previous
"""BP LDPC decoder as an on-chip PE-routed Trainium2 kernel.

The reference multiplies dense [E,E] (E=3456) "exclusive sum" operators every
iteration.  Those operators are the check-node / variable-node exclusive sums
of a (DV=3)-regular LDPC graph.  The variable-node side is local to a
partition-major edge layout; the check-node side (a fixed graph permutation +
segmented reduce) is done entirely on the tensor engine with 0/1 routing
matrices held in SBUF:

  scatter: tot[pc, g, :] = sum_s A_s^T @ (pair[:, s] masked into its g slot)
  gather:  U[pe, s, g, :] = B_s^T @ tot ; G = sum_g U * mask

so nothing round-trips through DRAM inside the iteration loop (the baseline
did 54 indirect DMAs per iteration through HBM; that was ~90% of its time).

The routed channels are fp16 (fp32 matmuls run 2 PE passes; fp16 runs one and
gets fast weight load).  The log-magnitude channel is carried as an fp16
hi/lo pair (lo = lt - fp16(lt)), so the reconstructed check sums match fp32
to ~2^-22 relative -- the decoded bits stay bit-exact vs the fp32 reference.
The sign-count channel is exact (small integers).

Sharding: pure data parallel over the batch (16 -> 2 samples per core); the
graph structure (routing matrices) is replicated.
"""

import sys
import numpy as np

for _p in ("/opt/trn_rl_repo", "/root/.axon_site/_ro/trn_rl_repo"):
    if _p not in sys.path:
        sys.path.insert(0, _p)

N = 1152          # variables
E = 3456          # edges (DV=3 per variable)
B = 16            # batch
NCORES = 8
BP = B // NCORES  # batch per core
NQ = N // 128     # variables per partition (9)
NS = E // 128     # edge slots per partition (27)
NG = 5            # check groups of 128 (576 checks -> 5*128 slots)
RC = 3            # routed channels: (lt_hi, lt_lo, b)
FW = RC * BP      # matmul free width per (s, g)

_A32 = float(np.float32(1.0 + 1e-8))
_B32 = float(np.float32(1.0 - 1e-8))


def _derive_structure(H_sumC_to_V, H_xe_v_sumc_to_y):
    """Recover the LDPC graph and build the PE routing operators.

    Device edge order is col-major (variable-major): e = 3*v + j, variable v
    on partition v // NQ, slot s = e % NS.  Check c sits at PSUM row c % 128
    in group c // 128.
    Returns:
      wA [128, NS, 128] f16: wA[pe, s, pc] = 1 iff edge (pe,s) -> check row pc
      wB [128, NS, 128] f16: wB[pc, s, pe] = wA[pe, s, pc]   (gather routing)
      mk [128, NS, NG] f32:  mk[pe, s, g]  = 1 iff edge (pe,s) -> group g
    """
    H_sumC_to_V = np.asarray(H_sumC_to_V)
    H_xe_v_sumc_to_y = np.asarray(H_xe_v_sumc_to_y)
    cols_rm = np.argmax(H_xe_v_sumc_to_y, axis=0)        # variable of row-major edge
    p_r2l = np.argsort(cols_rm, kind="stable")           # col-major pos -> row-major idx
    p_l2r = np.argsort(p_r2l)
    Br = H_sumC_to_V[:, p_l2r]                           # same-check adjacency (row-major)
    same = Br[np.arange(E - 1), np.arange(1, E)] > 0
    check_id = np.concatenate([[0], np.cumsum(~same)]).astype(np.int64)
    ck = check_id[p_r2l]                                 # check of device edge e
    assert ck.max() < NG * 128
    pc = (ck % 128).astype(np.int64).reshape(128, NS)    # [pe, s]
    g = (ck // 128).astype(np.int64).reshape(128, NS)
    pe_idx = np.arange(128)[:, None].repeat(NS, 1)
    s_idx = np.arange(NS)[None, :].repeat(128, 0)
    wA = np.zeros((128, NS, 128), np.float16)
    wA[pe_idx, s_idx, pc] = 1.0
    wB = np.zeros((128, NS, 128), np.float16)
    wB[pc, s_idx, pe_idx] = 1.0
    mk = np.zeros((128, NS, 1, NG), np.float32)
    mk[pe_idx, s_idx, 0, g] = 1.0
    mkb = np.zeros((128, NS, NG, RC, BP), np.float16)
    mkb[pe_idx, s_idx, g] = 1.0
    return wA, wB, mk, mkb


def _build_program(n_iters: int):
    import concourse.bacc as bacc
    import concourse.hw_specs as hw_specs
    # Force every activation onto the one table set containing all our
    # functions (ln/exp/sign/abs); otherwise the chooser ping-pongs between
    # sets and reloads tables (~2.7us per reload).
    if not getattr(bacc, "_act_tables_pinned", False):
        _orig_get = hw_specs.get_activation_tables

        def _pinned(arch):
            tabs = _orig_get(arch)
            keep = "natural_log_exp_and_others"
            if keep in tabs:
                tabs = {k: (v if k == keep else set()) for k, v in tabs.items()}
            return tabs

        bacc.get_activation_tables = _pinned
        bacc._act_tables_pinned = True
    import concourse.mybir as mybir
    import concourse.tile as tile

    f32 = mybir.dt.float32
    f16 = mybir.dt.float16
    i32 = mybir.dt.int32
    AF = mybir.ActivationFunctionType
    ALU = mybir.AluOpType

    nc = bacc.Bacc("TRN2", target_bir_lowering=False, debug=False)

    llr_d = nc.declare_dram_parameter("llr", [BP, N], f32, isOutput=False)
    wA_d = nc.declare_dram_parameter("wA", [128, NS * 128], f16, isOutput=False)
    wB_d = nc.declare_dram_parameter("wB", [128, NS * 128], f16, isOutput=False)
    mkb_d = nc.declare_dram_parameter("mkb", [128, NS * NG * RC * BP], f16,
                                      isOutput=False)
    dec_d = nc.declare_dram_parameter("dec", [BP, N], i32, isOutput=True)

    with tile.TileContext(nc) as tc:
        with tc.tile_pool(name="st", bufs=1) as st, \
             tc.tile_pool(name="ps", bufs=1, space="PSUM") as ps:
            wA = st.tile([128, NS, 128], f16)
            wB = st.tile([128, NS, 128], f16)
            mk = st.tile([128, NS, 1, NG], f32)
            mkb = st.tile([128, NS, NG, RC, BP], f16)  # pre-broadcast fp16 mask
            llr_sb = st.tile([128, NQ, BP], f32)
            x = st.tile([128, NQ, 3, BP], f32)         # per-edge v->c messages
            lt_f = st.tile([128, NS, BP], f32)         # fp32 log-magnitude
            pair = st.tile([128, NS, RC, BP], f16)     # (lt_hi, lt_lo, b) fp16
            R = st.tile([128, NS, NG, RC, BP], f16)    # mask-expanded rhs
            tbf = st.tile([128, RC, BP, NG], f16)      # gather rhs (hi, lo, K)
            Lt_f = st.tile([128, NG, BP], f32)
            Um = st.tile([128, NS, RC, BP, NG], f32)
            G3 = st.tile([128, NS, RC, BP], f32)       # gathered (hi, lo, K) sums
            S = st.tile([128, NQ, 3, BP], f32)
            dlt = st.tile([128, NQ, 3, BP], f32)
            db = st.tile([128, NQ, 3, BP], f32)
            lden = st.tile([128, NQ, 3, BP], f32)
            xp = st.tile([128, NQ, 3, BP], f32)
            ki = st.tile([128, NQ, 3, BP], i32)
            kb = st.tile([128, NQ, 3, BP], i32)
            sgnx = st.tile([128, NQ, 3, BP], f32)
            w = st.tile([128, NQ, 3, BP], f32)
            pd = st.tile([128, NQ, 3, BP], f32)
            L1 = st.tile([128, NQ, 3, BP], f32)
            L2 = st.tile([128, NQ, 3, BP], f32)
            yv = st.tile([128, NQ, 4, BP], f32)        # slots 0..2 = y_j, 3 = llr
            Vf = st.tile([128, NQ, BP], f32)           # llr + sum_j y_j
            sg = st.tile([128, NQ, BP], f32)
            dec_f = st.tile([128, NQ, BP], f32)
            dec_i = st.tile([128, NQ, BP], i32)

            tot_ps = ps.tile([128, NG, RC, BP], f32)   # 120B -> 1 bank
            # gather output in THREE tiles so the mask-select starts while the
            # PE is still filling later tiles (deps are per-tile)
            U_a = ps.tile([128, 10, 32], f32)          # s 0..9
            U_b = ps.tile([128, 9, 32], f32)           # s 10..18
            U_c = ps.tile([128, NS - 19, 32], f32)     # s 19..26
            ax_ps = ps.tile([128, NQ, 3, BP], f32)
            u_ps = ps.tile([128, NQ, 3, BP], f32)
            ln_ps = ps.tile([128, NQ, 3, BP], f32)

            # ---- init (ordered by when iteration 0 needs each tensor) ----
            nc.sync.dma_start(
                out=llr_sb[:], in_=llr_d.ap().rearrange("b (p q) -> p q b", p=128)
            )
            wA_ap = wA_d.ap().rearrange("p (s c) -> p s c", s=NS)
            wB_ap = wB_d.ap().rearrange("p (s c) -> p s c", s=NS)
            nc.sync.dma_start(out=mkb[:], in_=mkb_d.ap())
            nc.sync.dma_start(out=wA[:, 0:14], in_=wA_ap[:, 0:14])
            nc.sync.dma_start(out=wA[:, 14:NS], in_=wA_ap[:, 14:NS])
            nc.sync.dma_start(out=wB[:, 0:14], in_=wB_ap[:, 0:14])
            nc.sync.dma_start(out=wB[:, 14:NS], in_=wB_ap[:, 14:NS])
            # f32 mask for the PSUM-side select, derived from the fp16 one
            nc.vector.tensor_copy(mk[:, :, 0, :], mkb[:, :, :, 0, 0])
            nc.vector.tensor_copy(
                x[:], llr_sb[:].unsqueeze(2).to_broadcast([128, NQ, 3, BP])
            )
            nc.vector.tensor_copy(yv[:, :, 3, :], llr_sb[:])

            pair_hi = pair[:, :, 0, :]
            pair_lo = pair[:, :, 1, :]
            pair_b = pair[:, :, 2, :]
            ltq = lt_f[:].rearrange("p (q j) b -> p q j b", q=NQ)

            for t in range(n_iters):
                # lt = ln(1e-8 + tanh(|x|/2)) computed exp/ln-only:
                #   u = exp(-|x|); lt = ln(A - B*u) - ln(1 + u)
                # (intermediates go through PSUM: ScalarE's faster port)
                nc.scalar.activation(ax_ps[:], x[:], AF.Abs)
                nc.scalar.activation(u_ps[:], ax_ps[:], AF.Exp, scale=-1.0)
                nc.scalar.activation(ln_ps[:], u_ps[:], AF.Ln, bias=_A32,
                                     scale=-_B32)
                nc.scalar.activation(lden[:], u_ps[:], AF.Ln, bias=1.0)
                nc.vector.tensor_tensor(ltq, ln_ps[:], lden[:], ALU.subtract)
                # fp16 hi/lo split of lt; b = 1 if x < 0 else 0
                nc.vector.tensor_copy(pair_hi, lt_f[:])
                nc.vector.tensor_tensor(pair_lo, lt_f[:], pair_hi, ALU.subtract)
                nc.vector.tensor_scalar(
                    pair_b.rearrange("p (q j) b -> p q j b", q=NQ),
                    x[:], 0.0, None, ALU.is_lt)

                # mask-expand the per-edge rows into their check-group slot
                # (two halves so the scatter matmuls start on half A early)
                for sl, sh in ((0, 14), (14, NS)):
                    nc.vector.tensor_tensor(
                        R[:, sl:sh],
                        pair[:, sl:sh].unsqueeze(2).to_broadcast(
                            [128, sh - sl, NG, RC, BP]),
                        mkb[:, sl:sh],
                        ALU.mult)
                # scatter: tot[pc, g] = sum over edges of check (pc, g)
                for s in range(NS):
                    nc.tensor.matmul(
                        tot_ps[:],
                        wA[:, s, :],
                        R[:, s].rearrange("p g r b -> p (g r b)"),
                        start=(s == 0), stop=(s == NS - 1),
                    )
                # rebuild fp16 hi/lo of the per-check sums for the gather pass
                nc.vector.tensor_reduce(
                    Lt_f[:], tot_ps[:, :, 0:2, :].transpose([0, 1, 3, 2]),
                    axis=mybir.AxisListType.X, op=ALU.add)
                nc.vector.tensor_copy(
                    tbf[:, 0, :, :], Lt_f[:].transpose([0, 2, 1]))
                nc.vector.tensor_tensor(
                    tbf[:, 1, :, :], Lt_f[:].transpose([0, 2, 1]),
                    tbf[:, 0, :, :], ALU.subtract)
                nc.vector.tensor_copy(
                    tbf[:, 2, :, :], tot_ps[:, :, 2, :].transpose([0, 2, 1]))
                # gather: U[pe, s, g] = tot[pc(pe,s), g]  (free order r, b, g)
                tot_flat = tbf[:].rearrange("p r b g -> p (r b g)")
                uparts = ((U_a, 0, 10), (U_b, 10, 19), (U_c, 19, NS))
                for Ut, sl, sh in uparts:
                    for s in range(sl, sh):
                        nc.tensor.matmul(
                            Ut[:, s - sl, 0:NG * FW], wB[:, s, :], tot_flat,
                            start=True, stop=True,
                        )
                Um4 = Um[:].rearrange("p s r b g -> p s (r b) g")
                for Ut, sl, sh in uparts:
                    nc.vector.tensor_tensor(
                        Um4[:, sl:sh],
                        Ut[:, :, 0:NG * FW].rearrange(
                            "p s (rb g) -> p s rb g", g=NG),
                        mk[:, sl:sh].to_broadcast([128, sh - sl, RC * BP, NG]),
                        ALU.mult)
                nc.vector.tensor_reduce(
                    G3[:], Um4, axis=mybir.AxisListType.X, op=ALU.add)

                # exclusive check sums: sr = Lt - lt, kx = K - b
                G3q = G3[:].rearrange("p (q j) r b -> p q j r b", q=NQ)
                nc.vector.tensor_tensor(S[:], G3q[:, :, :, 0, :], G3q[:, :, :, 1, :],
                                        ALU.add)
                nc.vector.tensor_tensor(dlt[:], S[:], ltq, ALU.subtract)
                nc.vector.tensor_tensor(
                    db[:], G3q[:, :, :, 2, :],
                    pair_b.rearrange("p (q j) b -> p q j b", q=NQ), ALU.subtract)
                nc.scalar.activation(xp[:], dlt[:], AF.Exp)
                # sign of exclusive product: (-1)^kx
                nc.vector.tensor_copy(ki[:], db[:])
                nc.vector.tensor_scalar(kb[:], ki[:], 1, None, ALU.bitwise_and)
                nc.vector.tensor_scalar(sgnx[:], kb[:], -2.0, 1.0, ALU.mult, ALU.add)
                # y = ln(1 + pd) - ln(1 - pd), pd = sgn * (min(xp, 1) - 2e-7)
                # (clamp: fp16 hi/lo transport noise can push xp past 1 +- the
                #  2e-7 guard; the exact product never exceeds 1)
                nc.vector.tensor_scalar(w[:], xp[:], 1.0, -2e-7, ALU.min, ALU.add)
                nc.vector.tensor_tensor(pd[:], w[:], sgnx[:], ALU.mult)
                nc.scalar.activation(L1[:], pd[:], AF.Ln, bias=1.0)
                nc.scalar.activation(L2[:], pd[:], AF.Ln, bias=1.0, scale=-1.0)
                nc.vector.tensor_tensor(yv[:, :, 0:3, :], L1[:], L2[:], ALU.subtract)

                # variable side is local: Vf = llr + sum_j y_j
                nc.vector.tensor_reduce(
                    Vf[:], yv[:].transpose([0, 1, 3, 2]),
                    axis=mybir.AxisListType.X, op=ALU.add)
                if t < n_iters - 1:
                    nc.vector.tensor_tensor(
                        x[:],
                        Vf[:].unsqueeze(2).to_broadcast([128, NQ, 3, BP]),
                        yv[:, :, 0:3, :], ALU.subtract)
                else:
                    nc.scalar.activation(sg[:], Vf[:], AF.Sign)
                    nc.vector.tensor_scalar(dec_f[:], sg[:], -0.5, 0.5,
                                            ALU.mult, ALU.add)
                    nc.vector.tensor_copy(dec_i[:], dec_f[:])
                    nc.sync.dma_start(
                        out=dec_d.ap().rearrange("b (p q) -> p q b", p=128),
                        in_=dec_i[:],
                    )
    nc.compile()
    return nc


_PROGRAM_CACHE = {}


def _get_program(n_iters: int):
    if n_iters not in _PROGRAM_CACHE:
        _PROGRAM_CACHE[n_iters] = _build_program(n_iters)
    return _PROGRAM_CACHE[n_iters]


def _make_in_maps(llr_in, H_sumC_to_V, H_xe_v_sumc_to_y):
    llr = np.ascontiguousarray(np.asarray(llr_in, dtype=np.float32))
    assert llr.shape == (B, N)
    wA, wB, mk, mkb = _derive_structure(H_sumC_to_V, H_xe_v_sumc_to_y)
    wA = np.ascontiguousarray(wA.reshape(128, NS * 128))
    wB = np.ascontiguousarray(wB.reshape(128, NS * 128))
    mkb = np.ascontiguousarray(mkb.reshape(128, NS * NG * RC * BP))
    return [
        {
            "llr": np.ascontiguousarray(llr[c * BP:(c + 1) * BP]),
            "wA": wA,
            "wB": wB,
            "mkb": mkb,
        }
        for c in range(NCORES)
    ]


def kernel(llr_in, H_x_to_xe0, H_sumC_to_V, H_sumV_to_C, H_xe_v_sumc_to_y,
           bp_iter_num, **_unused):
    from concourse.bass_utils import run_bass_kernel_spmd

    n_iters = int(np.asarray(bp_iter_num))
    nc = _get_program(n_iters)
    in_maps = _make_in_maps(llr_in, H_sumC_to_V, H_xe_v_sumc_to_y)
    res = run_bass_kernel_spmd(nc, in_maps, list(range(NCORES)))
    out = np.concatenate([res.results[c]["dec"] for c in range(NCORES)], axis=0)
    return out.astype(np.int32)
